# revision 2
# baseline (speedup 1.0000x reference)
"""Trainium2 8-core kernel for the Contrast module:

    za_p = ELU(za @ W1 + b1) @ W2 + b2          (same for zb)
    za_ca = softmax((za_p Wq + bq)(zb_p Wk + bk)^T / sqrt(256)) @ (zb_p Wv + bv)
    zb_ca = softmax((zb_p Wq + bq)(za_p Wk + bk)^T / sqrt(256)) @ (za_p Wv + bv)
    out = concat(za_ca, zb_ca, axis=1)

Sharding: rows (N axis) data-parallel across 8 cores; weights replicated.
Each core projects its 1024-row slice of za and zb, computes its K/V
shards, AllGathers K/V (2 collectives, overlapped with compute), and runs
its [1024, 8192] attention block for both directions.

Layout notes:
 - All activations flow feature-major ("transposed"): inputs arrive as
   zaT [h, n] so every matmul contracts over the partition axis without
   any on-chip transposes.  out = lhsT.T @ rhs with
     hT = W1^T zaT, pT = W2^T hT, QT/KT = W^T pT (feature-major)
     V  = pT as lhsT with Wv as rhs (token-major)
     scoresT[k, q] = (KT slice)^T @ QT, attn@V = expT slice as lhsT, V as rhs.
 - f32r (FP32 data, FP22 multiply) matmuls: full PE rate at moving dim>=256.
 - softmax denominator: V panels carry a ones column; attn@V is split into
   N=256 and N=258 matmuls so the rowsum accumulates in PSUM col 512.
 - No max-subtraction in softmax: scores are ~N(0, 0.85^2), exp is safe.
 - ELU+1 = max(x+1, min(exp(x), 1)); the -1 is folded into b2 on the host.
 - 1/16 score scale folded into Wq/bq on the host.
"""

import numpy as np

import concourse.mybir as mybir
import concourse.tile as tile
from concourse import bacc
from concourse.bass_utils import run_bass_kernel_spmd

dt = mybir.dt
AF = mybir.ActivationFunctionType
ALU = mybir.AluOpType

R = 8            # cores
N = 8192         # total rows
H = 1024         # hidden
D = 512          # attention dim
NL = N // R      # rows per core
HC = H // 128    # 8 h-chunks
DC = D // 128    # 4 d-chunks
NB = NL // 512   # 2 n-blocks per core slice
SCALE = 16.0     # sqrt(512/2)
KVF = D * NL     # floats per K (or V) shard

F32R = dt.float32r


def _r(ap):
    return ap.bitcast(F32R)


def build():
    nc = bacc.Bacc("TRN2", target_bir_lowering=False, debug=False, num_devices=R)

    def inp(name, shape):
        return nc.dram_tensor(name, shape, dt.float32, kind="ExternalInput")

    zT = {"a": inp("zaT", [128, HC, NL]), "b": inp("zbT", [128, HC, NL])}
    w1 = inp("W1t", [128, HC, H])
    w2 = inp("W2t", [128, HC, D])
    wq = inp("Wqt", [128, DC, D])
    wk = inp("Wkt", [128, DC, D])
    wv = inp("Wvt", [128, DC, D])
    b1d = inp("b1t", [128, HC])
    b1p1d = inp("b1p1t", [128, HC])
    b2d = inp("b2t", [128, DC])
    bqd = inp("bqt", [128, DC])
    bkd = inp("bkt", [128, DC])
    bvd = inp("bvt", [128, D])
    vpadd = inp("vpad", [128, 2 * HC])
    out_d = nc.dram_tensor("out", [NL, 2 * D], dt.float32, kind="ExternalOutput")

    with tile.TileContext(nc) as tc:
        psum = tc.alloc_tile_pool(name="psum", bufs=1, space="PSUM")
        dram = tc.alloc_tile_pool(name="dram", bufs=1, space="DRAM")
        const = tc.alloc_tile_pool(name="const", bufs=1)
        qtp = tc.alloc_tile_pool(name="qtp", bufs=1)
        wkvp = tc.alloc_tile_pool(name="wkvp", bufs=1)
        projp = tc.alloc_tile_pool(name="projp", bufs=1)

        # ---- constants ----
        b1 = const.tile([128, HC], dt.float32, name="b1")
        b1p1 = const.tile([128, HC], dt.float32, name="b1p1")
        b2 = const.tile([128, DC], dt.float32, name="b2")
        bq = const.tile([128, DC], dt.float32, name="bq")
        bk = const.tile([128, DC], dt.float32, name="bk")
        bv = const.tile([128, D], dt.float32, name="bv")
        for t, d_ in ((b1, b1d), (b1p1, b1p1d), (b2, b2d), (bq, bqd), (bk, bkd), (bv, bvd)):
            nc.sync.dma_start(t[:], d_.ap())
        wqt = wkvp.tile([128, DC, D], F32R, name="wqt")
        wkt = wkvp.tile([128, DC, D], F32R, name="wkt")
        wvt = wkvp.tile([128, DC, D], F32R, name="wvt")
        for t, d_ in ((wqt, wq), (wkt, wk), (wvt, wv)):
            nc.sync.dma_start(t[:], _r(d_.ap()))

        # ---- projection weights (chunked DMA so PE can start early) ----
        w1t = projp.tile([128, HC, H], F32R, name="w1t")
        for hc in range(HC):
            nc.sync.dma_start(w1t[:, hc, :], _r(w1.ap()[:, hc, :]))
        w2t = projp.tile([128, HC, D], F32R, name="w2t")
        nc.sync.dma_start(w2t[:], _r(w2.ap()))

        pT = {
            "a": wkvp.tile([128, DC, NL], F32R, name="pta"),
            "b": wkvp.tile([128, DC, NL], F32R, name="ptb"),
        }

        # AG buffers: direction X's attention consumes K/V derived from the
        # OTHER projection; ag_for[X] is filled from pT[other(X)].
        agin = {}
        agout = {}
        for x in ("b", "a"):
            agin[x] = dram.tile([2 * KVF], dt.float32, name=f"agin_{x}")
            agout[x] = dram.tile(
                [R * 2 * KVF], dt.float32, name=f"agout_{x}", addr_space="Shared"
            )

        # ================= projection + K/V shards =================
        for src, other in (("a", "b"), ("b", "a")):
            for nb in range(NB):
                ns = slice(nb * 512, (nb + 1) * 512)
                z = projp.tile([128, HC, 512], F32R, tag="z", bufs=2, name=f"z_{src}{nb}")
                nc.sync.dma_start(z[:], _r(zT[src].ap()[:, :, ns]))
                hT = projp.tile([128, HC, 512], F32R, tag="h", bufs=1, name=f"h_{src}{nb}")
                for d1c in range(HC):
                    ps = psum.tile([128, 512], dt.float32, tag="mm", bufs=3, name="ps_h")
                    for hc in range(HC):
                        nc.tensor.matmul(
                            ps[:],
                            w1t[:, hc, d1c * 128 : (d1c + 1) * 128],
                            z[:, hc, :],
                            start=(hc == 0),
                            stop=(hc == HC - 1),
                        )
                    # ELU(x)+1 = max(x+1, min(exp(x), 1)), x = ps + b1
                    e = projp.tile([128, 512], dt.float32, tag="e", bufs=2, name="e")
                    nc.scalar.activation(e[:], ps[:], AF.Exp, bias=b1[:, d1c : d1c + 1])
                    xp1 = projp.tile([128, 512], dt.float32, tag="xp1", bufs=2, name="xp1")
                    nc.vector.tensor_scalar(
                        xp1[:], ps[:], b1p1[:, d1c : d1c + 1], None, ALU.add
                    )
                    nc.vector.tensor_scalar(e[:], e[:], 1.0, None, ALU.min)
                    nc.vector.tensor_tensor(hT[:, d1c, :], xp1[:], e[:], ALU.max)
                for d2c in range(DC):
                    ps = psum.tile([128, 512], dt.float32, tag="mm", bufs=3, name="ps_p")
                    for d1c in range(HC):
                        nc.tensor.matmul(
                            ps[:],
                            w2t[:, d1c, d2c * 128 : (d2c + 1) * 128],
                            hT[:, d1c, :],
                            start=(d1c == 0),
                            stop=(d1c == HC - 1),
                        )
                    nc.scalar.activation(
                        pT[src][:, d2c, ns], ps[:], AF.Identity, bias=b2[:, d2c : d2c + 1]
                    )

            # K/V shards for the *other* direction, written to AG input
            ktv = agin[other][0:KVF].rearrange("(d n) -> d n", n=NL)
            vv = agin[other][KVF : 2 * KVF].rearrange("(n d) -> n d", d=D)
            for dc in range(DC):
                for nb in range(NB):
                    ps = psum.tile([128, 512], dt.float32, tag="mm", bufs=3, name="ps_k")
                    for d2c in range(DC):
                        nc.tensor.matmul(
                            ps[:],
                            wkt[:, d2c, dc * 128 : (dc + 1) * 128],
                            pT[src][:, d2c, nb * 512 : (nb + 1) * 512],
                            start=(d2c == 0),
                            stop=(d2c == DC - 1),
                        )
                    s = projp.tile([128, 512], dt.float32, tag="stg", bufs=3, name="stg_k")
                    nc.scalar.activation(s[:], ps[:], AF.Identity, bias=bk[:, dc : dc + 1])
                    nc.sync.dma_start(
                        ktv[dc * 128 : (dc + 1) * 128, nb * 512 : (nb + 1) * 512], s[:]
                    )
            for nt in range(NL // 128):
                ps = psum.tile([128, 512], dt.float32, tag="mm", bufs=3, name="ps_v")
                for d2c in range(DC):
                    nc.tensor.matmul(
                        ps[:],
                        pT[src][:, d2c, nt * 128 : (nt + 1) * 128],
                        wvt[:, d2c, :],
                        start=(d2c == 0),
                        stop=(d2c == DC - 1),
                    )
                s = projp.tile([128, 512], dt.float32, tag="stg", bufs=3, name="stg_v")
                nc.scalar.activation(s[:], ps[:], AF.Copy)
                nc.sync.dma_start(vv[nt * 128 : (nt + 1) * 128, :], s[:])
            nc.gpsimd.collective_compute(
                "AllGather",
                ALU.bypass,
                ins=[agin[other].opt()],
                outs=[agout[other].opt()],
                replica_groups=[list(range(R))],
            )

        projp.release()

        # ================= queries =================
        qT = {}
        for x in ("b", "a"):
            qT[x] = qtp.tile([128, DC, NL], F32R, name=f"qt_{x}")
            for dc in range(DC):
                for nb in range(NB):
                    ps = psum.tile([128, 512], dt.float32, tag="mm", bufs=3, name="ps_q")
                    for d2c in range(DC):
                        nc.tensor.matmul(
                            ps[:],
                            wqt[:, d2c, dc * 128 : (dc + 1) * 128],
                            pT[x][:, d2c, nb * 512 : (nb + 1) * 512],
                            start=(d2c == 0),
                            stop=(d2c == DC - 1),
                        )
                    nc.scalar.activation(
                        qT[x][:, dc, nb * 512 : (nb + 1) * 512],
                        ps[:],
                        AF.Identity,
                        bias=bq[:, dc : dc + 1],
                    )
        wkvp.release()

        # ================= attention =================
        attnp = tc.alloc_tile_pool(name="attnp", bufs=1)
        for x, col in (("b", 1), ("a", 0)):
            accs = {}
            for r in range(R):
                base = r * 2 * KVF
                ktile = attnp.tile([128, DC, NL], F32R, tag="kt", bufs=2, name=f"kt{r}")
                nc.sync.dma_start(
                    ktile[:],
                    _r(
                        agout[x][base : base + KVF].rearrange(
                            "(dc p n) -> p dc n", p=128, n=NL
                        )
                    ),
                )
                vtile = attnp.tile(
                    [128, NL // 128, D + 2], F32R, tag="vt", bufs=2, name=f"vt{r}"
                )
                nc.sync.dma_start(
                    vtile[:, :, 0:D],
                    _r(
                        agout[x][base + KVF : base + 2 * KVF].rearrange(
                            "(kc p d) -> p kc d", p=128, d=D
                        )
                    ),
                )
                nc.sync.dma_start(
                    vtile[:, :, D : D + 2],
                    _r(vpadd.ap().rearrange("p (kc c) -> p kc c", c=2)),
                )
                for qb in range(NB):
                    qs = slice(qb * 512, (qb + 1) * 512)
                    exps = []
                    for kt_i in range(NL // 128):
                        ps = psum.tile(
                            [128, 512], dt.float32, tag="mm", bufs=3, name="ps_s"
                        )
                        for dc in range(DC):
                            nc.tensor.matmul(
                                ps[:],
                                ktile[:, dc, kt_i * 128 : (kt_i + 1) * 128],
                                qT[x][:, dc, qs],
                                start=(dc == 0),
                                stop=(dc == DC - 1),
                            )
                        ex = attnp.tile(
                            [128, 512], F32R, tag="exp", bufs=12, name=f"ex{kt_i}"
                        )
                        nc.scalar.activation(ex[:], ps[:], AF.Exp)
                        exps.append(ex)
                    for qt_i in range(4):
                        qsl = slice(qt_i * 128, (qt_i + 1) * 128)
                        p1 = psum.tile(
                            [128, 256], dt.float32, tag="po1", bufs=2, name="po1"
                        )
                        p2 = psum.tile(
                            [128, 258], dt.float32, tag="po2", bufs=2, name="po2"
                        )
                        for kc in range(NL // 128):
                            nc.tensor.matmul(
                                p1[:],
                                exps[kc][:, qsl],
                                vtile[:, kc, 0:256],
                                start=(kc == 0),
                                stop=(kc == NL // 128 - 1),
                            )
                            nc.tensor.matmul(
                                p2[:],
                                exps[kc][:, qsl],
                                vtile[:, kc, 256 : D + 2],
                                start=(kc == 0),
                                stop=(kc == NL // 128 - 1),
                            )
                        if r == 0:
                            acc = attnp.tile(
                                [128, D + 2], dt.float32, tag="acc", bufs=8,
                                name=f"acc{qb}{qt_i}",
                            )
                            accs[(qb, qt_i)] = acc
                            nc.vector.tensor_copy(acc[:, 0:256], p1[:])
                            nc.vector.tensor_copy(acc[:, 256 : D + 2], p2[:])
                        else:
                            acc = accs[(qb, qt_i)]
                            nc.vector.tensor_tensor(
                                acc[:, 0:256], acc[:, 0:256], p1[:], ALU.add
                            )
                            nc.vector.tensor_tensor(
                                acc[:, 256 : D + 2], acc[:, 256 : D + 2], p2[:], ALU.add
                            )
            # finalize: out = acc[:, :512] / acc[:, 512] + bv
            for qb in range(NB):
                for qt_i in range(4):
                    acc = accs[(qb, qt_i)]
                    rr = attnp.tile([128, 1], dt.float32, tag="rr", bufs=4, name="rr")
                    nc.vector.reciprocal(rr[:], acc[:, D : D + 1])
                    ot = attnp.tile([128, D], dt.float32, tag="ot", bufs=3, name="ot")
                    nc.vector.tensor_scalar(ot[:], acc[:, 0:D], rr[:], None, ALU.mult)
                    nc.vector.tensor_tensor(ot[:], ot[:], bv[:], ALU.add)
                    r0 = qb * 512 + qt_i * 128
                    nc.sync.dma_start(
                        out_d.ap()[r0 : r0 + 128, col * D : (col + 1) * D], ot[:]
                    )
        attnp.release()
        qtp.release()
        const.release()
        dram.release()
        psum.release()

    nc.compile()
    return nc


_NC = None


def _get_nc():
    global _NC
    if _NC is None:
        _NC = build()
    return _NC


def _chunk_w(w):
    """[X, Y] -> [128, X//128, Y] partition-chunked, contiguous."""
    x, y = w.shape
    return np.ascontiguousarray(w.reshape(x // 128, 128, y).transpose(1, 0, 2))


def _chunk_b(b):
    return np.ascontiguousarray(np.asarray(b, np.float32).reshape(-1, 128).T)


def prep_in_maps(za, zb, W1, b1, W2, b2, Wq, bq, Wk, bk, Wv, bv):
    za = np.asarray(za, np.float32)
    zb = np.asarray(zb, np.float32)
    W1 = np.asarray(W1, np.float32)
    W2 = np.asarray(W2, np.float32)
    Wq = np.asarray(Wq, np.float32)
    Wk = np.asarray(Wk, np.float32)
    Wv = np.asarray(Wv, np.float32)
    b1 = np.asarray(b1, np.float32)
    b2 = np.asarray(b2, np.float32)
    bq = np.asarray(bq, np.float32)
    bk = np.asarray(bk, np.float32)
    bv = np.asarray(bv, np.float32)

    shared = {
        "W1t": _chunk_w(W1),
        "W2t": _chunk_w(W2),
        "Wqt": _chunk_w(Wq / SCALE),
        "Wkt": _chunk_w(Wk),
        "Wvt": _chunk_w(Wv),
        "b1t": _chunk_b(b1),
        "b1p1t": _chunk_b(b1 + 1.0),
        "b2t": _chunk_b(b2 - W2.sum(axis=0)),
        "bqt": _chunk_b(bq / SCALE),
        "bkt": _chunk_b(bk),
        "bvt": np.ascontiguousarray(np.broadcast_to(bv, (128, D)).astype(np.float32)),
        "vpad": np.ascontiguousarray(
            np.broadcast_to(np.tile(np.array([1.0, 0.0], np.float32), HC), (128, 2 * HC))
        ),
    }
    zaT = np.ascontiguousarray(za.T)  # [H, N]
    zbT = np.ascontiguousarray(zb.T)
    in_maps = []
    for c in range(R):
        cs = slice(c * NL, (c + 1) * NL)
        in_maps.append(
            {
                "zaT": _chunk_w(zaT[:, cs]),
                "zbT": _chunk_w(zbT[:, cs]),
                **shared,
            }
        )
    return in_maps


def kernel(**inputs) -> np.ndarray:
    nc = _get_nc()
    in_maps = prep_in_maps(**inputs)
    res = run_bass_kernel_spmd(nc, in_maps, core_ids=list(range(R)))
    return np.concatenate([res.results[c]["out"] for c in range(R)], axis=0)


# revision 3
# speedup vs baseline: 1.2556x; 1.2556x over previous
"""Trainium2 8-core kernel for the Contrast module:

    za_p = ELU(za @ W1 + b1) @ W2 + b2          (same for zb)
    za_ca = softmax((za_p Wq + bq)(zb_p Wk + bk)^T / sqrt(256)) @ (zb_p Wv + bv)
    zb_ca = softmax((zb_p Wq + bq)(za_p Wk + bk)^T / sqrt(256)) @ (za_p Wv + bv)
    out = concat(za_ca, zb_ca, axis=1)

Sharding: rows (N axis) data-parallel across 8 cores; weights replicated.
Each core projects its 1024-row slice of za and zb, computes its K/V
shards, AllGathers K/V (2 collectives, overlapped with compute), and runs
its [1024, 8192] attention block for both directions.

Layout notes:
 - All activations flow feature-major ("transposed"): inputs arrive as
   zaT [h, n] so every matmul contracts over the partition axis without
   any on-chip transposes.  out = lhsT.T @ rhs with
     hT = W1^T zaT, pT = W2^T hT, QT/KT = W^T pT (feature-major)
     V  = pT as lhsT with Wv as rhs (token-major)
     scoresT[k, q] = (KT slice)^T @ QT, attn@V = expT slice as lhsT, V as rhs.
 - f32r (FP32 data, FP22 multiply) matmuls: full PE rate at moving dim>=256.
 - softmax denominator: V panels carry a ones column; attn@V is split into
   N=256 and N=258 matmuls so the rowsum accumulates in PSUM col 512.
 - No max-subtraction in softmax: scores are ~N(0, 0.85^2), exp is safe.
 - ELU+1 = max(x+1, min(exp(x), 1)); the -1 is folded into b2 on the host.
 - 1/16 score scale folded into Wq/bq on the host.
"""

import numpy as np

import concourse.mybir as mybir
import concourse.tile as tile
from concourse import bacc
from concourse.bass_utils import run_bass_kernel_spmd

dt = mybir.dt
AF = mybir.ActivationFunctionType
ALU = mybir.AluOpType

R = 8            # cores
N = 8192         # total rows
H = 1024         # hidden
D = 512          # attention dim
NL = N // R      # rows per core
HC = H // 128    # 8 h-chunks
DC = D // 128    # 4 d-chunks
NB = NL // 512   # 2 n-blocks per core slice
SCALE = 16.0     # sqrt(512/2)
KVF = D * NL     # floats per K (or V) shard

F32R = dt.float32r


def _r(ap):
    return ap.bitcast(F32R)


def build():
    nc = bacc.Bacc("TRN2", target_bir_lowering=False, debug=False, num_devices=R)

    def inp(name, shape):
        return nc.dram_tensor(name, shape, dt.float32, kind="ExternalInput")

    zT = {"a": inp("zaT", [128, HC, NL]), "b": inp("zbT", [128, HC, NL])}
    w1 = inp("W1t", [128, HC, H])
    w2 = inp("W2t", [128, HC, D])
    wq = inp("Wqt", [128, DC, D])
    wk = inp("Wkt", [128, DC, D])
    wv = inp("Wvt", [128, DC, D])
    b1d = inp("b1t", [128, HC])
    b1p1d = inp("b1p1t", [128, HC])
    b2d = inp("b2t", [128, DC])
    bqd = inp("bqt", [128, DC])
    bkd = inp("bkt", [128, DC])
    bvd = inp("bvt", [128, D])
    vpadd = inp("vpad", [128, 2 * HC])
    out_d = nc.dram_tensor("out", [NL, 2 * D], dt.float32, kind="ExternalOutput")

    with tile.TileContext(nc) as tc:
        psum = tc.alloc_tile_pool(name="psum", bufs=1, space="PSUM")
        dram = tc.alloc_tile_pool(name="dram", bufs=1, space="DRAM")
        const = tc.alloc_tile_pool(name="const", bufs=1)
        qtp = tc.alloc_tile_pool(name="qtp", bufs=1)
        wkvp = tc.alloc_tile_pool(name="wkvp", bufs=1)
        projp = tc.alloc_tile_pool(name="projp", bufs=1)

        # ---- constants ----
        b1 = const.tile([128, HC], dt.float32, name="b1")
        b1p1 = const.tile([128, HC], dt.float32, name="b1p1")
        b2 = const.tile([128, DC], dt.float32, name="b2")
        bq = const.tile([128, DC], dt.float32, name="bq")
        bk = const.tile([128, DC], dt.float32, name="bk")
        bv = const.tile([128, D], dt.float32, name="bv")
        for t, d_ in ((b1, b1d), (b1p1, b1p1d), (b2, b2d), (bq, bqd), (bk, bkd), (bv, bvd)):
            nc.sync.dma_start(t[:], d_.ap())
        wqt = wkvp.tile([128, DC, D], F32R, name="wqt")
        wkt = wkvp.tile([128, DC, D], F32R, name="wkt")
        wvt = wkvp.tile([128, DC, D], F32R, name="wvt")
        for t, d_ in ((wqt, wq), (wkt, wk), (wvt, wv)):
            nc.sync.dma_start(t[:], _r(d_.ap()))

        # ---- projection weights (chunked DMA so PE can start early) ----
        w1t = projp.tile([128, HC, H], F32R, name="w1t")
        for hc in range(HC):
            nc.sync.dma_start(w1t[:, hc, :], _r(w1.ap()[:, hc, :]))
        w2t = projp.tile([128, HC, D], F32R, name="w2t")
        nc.sync.dma_start(w2t[:], _r(w2.ap()))

        pT = {
            "a": wkvp.tile([128, DC, NL], F32R, name="pta"),
            "b": wkvp.tile([128, DC, NL], F32R, name="ptb"),
        }

        # AG buffers: direction X's attention consumes K/V derived from the
        # OTHER projection; ag_for[X] is filled from pT[other(X)].
        agin_k = {}
        agin_v = {}
        agout_k = {}
        agout_v = {}
        for x in ("b", "a"):
            agin_k[x] = dram.tile([KVF], dt.float32, name=f"agink_{x}")
            agin_v[x] = dram.tile([KVF], dt.float32, name=f"aginv_{x}")
            agout_k[x] = dram.tile(
                [R * KVF], dt.float32, name=f"agoutk_{x}", addr_space="Shared"
            )
            agout_v[x] = dram.tile(
                [R * KVF], dt.float32, name=f"agoutv_{x}", addr_space="Shared"
            )

        # ================= projection + K/V shards =================
        for src, other in (("a", "b"), ("b", "a")):
            for nb in range(NB):
                ns = slice(nb * 512, (nb + 1) * 512)
                z = projp.tile([128, HC, 512], F32R, tag="z", bufs=2, name=f"z_{src}{nb}")
                nc.sync.dma_start(z[:], _r(zT[src].ap()[:, :, ns]))
                hT = projp.tile([128, HC, 512], F32R, tag="h", bufs=1, name=f"h_{src}{nb}")
                for d1c in range(HC):
                    ps = psum.tile([128, 512], dt.float32, tag="mm", bufs=4, name="ps_h")
                    for hc in range(HC):
                        nc.tensor.matmul(
                            ps[:],
                            w1t[:, hc, d1c * 128 : (d1c + 1) * 128],
                            z[:, hc, :],
                            start=(hc == 0),
                            stop=(hc == HC - 1),
                        )
                    # ELU(x)+1 = max(x+1, min(exp(x), 1)), x = ps + b1
                    e = projp.tile([128, 512], dt.float32, tag="e", bufs=2, name="e")
                    nc.scalar.activation(e[:], ps[:], AF.Exp, bias=b1[:, d1c : d1c + 1])
                    xp1 = projp.tile([128, 512], dt.float32, tag="xp1", bufs=2, name="xp1")
                    nc.vector.tensor_scalar(
                        xp1[:], ps[:], b1p1[:, d1c : d1c + 1], None, ALU.add
                    )
                    nc.vector.tensor_scalar(e[:], e[:], 1.0, None, ALU.min)
                    nc.vector.tensor_tensor(hT[:, d1c, :], xp1[:], e[:], ALU.max)
                for d2c in range(DC):
                    ps = psum.tile([128, 512], dt.float32, tag="mm", bufs=4, name="ps_p")
                    for d1c in range(HC):
                        nc.tensor.matmul(
                            ps[:],
                            w2t[:, d1c, d2c * 128 : (d2c + 1) * 128],
                            hT[:, d1c, :],
                            start=(d1c == 0),
                            stop=(d1c == HC - 1),
                        )
                    nc.scalar.activation(
                        pT[src][:, d2c, ns], ps[:], AF.Identity, bias=b2[:, d2c : d2c + 1]
                    )

            # K/V shards for the *other* direction, written to AG input
            ktv = agin_k[other][:].rearrange("(d n) -> d n", n=NL)
            vv = agin_v[other][:].rearrange("(n d) -> n d", d=D)
            for dc in range(DC):
                for nb in range(NB):
                    ps = psum.tile([128, 512], dt.float32, tag="mm", bufs=4, name="ps_k")
                    for d2c in range(DC):
                        nc.tensor.matmul(
                            ps[:],
                            wkt[:, d2c, dc * 128 : (dc + 1) * 128],
                            pT[src][:, d2c, nb * 512 : (nb + 1) * 512],
                            start=(d2c == 0),
                            stop=(d2c == DC - 1),
                        )
                    s = projp.tile([128, 512], dt.float32, tag="stg", bufs=3, name="stg_k")
                    nc.scalar.activation(s[:], ps[:], AF.Identity, bias=bk[:, dc : dc + 1])
                    nc.sync.dma_start(
                        ktv[dc * 128 : (dc + 1) * 128, nb * 512 : (nb + 1) * 512], s[:]
                    )
            nc.gpsimd.collective_compute(
                "AllGather",
                ALU.bypass,
                ins=[agin_k[other].opt()],
                outs=[agout_k[other].opt()],
                replica_groups=[list(range(R))],
            )
            for nt in range(NL // 128):
                ps = psum.tile([128, 512], dt.float32, tag="mm", bufs=4, name="ps_v")
                for d2c in range(DC):
                    nc.tensor.matmul(
                        ps[:],
                        pT[src][:, d2c, nt * 128 : (nt + 1) * 128],
                        wvt[:, d2c, :],
                        start=(d2c == 0),
                        stop=(d2c == DC - 1),
                    )
                s = projp.tile([128, 512], dt.float32, tag="stg", bufs=3, name="stg_v")
                nc.scalar.activation(s[:], ps[:], AF.Copy)
                nc.sync.dma_start(vv[nt * 128 : (nt + 1) * 128, :], s[:])
            nc.gpsimd.collective_compute(
                "AllGather",
                ALU.bypass,
                ins=[agin_v[other].opt()],
                outs=[agout_v[other].opt()],
                replica_groups=[list(range(R))],
            )

        projp.release()

        # ================= queries =================
        qT = {}
        for x in ("b", "a"):
            qT[x] = qtp.tile([128, DC, NL], F32R, name=f"qt_{x}")
            for dc in range(DC):
                for nb in range(NB):
                    ps = psum.tile([128, 512], dt.float32, tag="mm", bufs=4, name="ps_q")
                    for d2c in range(DC):
                        nc.tensor.matmul(
                            ps[:],
                            wqt[:, d2c, dc * 128 : (dc + 1) * 128],
                            pT[x][:, d2c, nb * 512 : (nb + 1) * 512],
                            start=(d2c == 0),
                            stop=(d2c == DC - 1),
                        )
                    nc.scalar.activation(
                        qT[x][:, dc, nb * 512 : (nb + 1) * 512],
                        ps[:],
                        AF.Identity,
                        bias=bq[:, dc : dc + 1],
                    )
        wkvp.release()

        # ================= attention =================
        attnp = tc.alloc_tile_pool(name="attnp", bufs=1)
        for x, col in (("b", 1), ("a", 0)):
            accs = {}
            for r in range(R):
                base = r * KVF
                ktile = attnp.tile([128, DC, NL], F32R, tag="kt", bufs=2, name=f"kt{r}")
                nc.sync.dma_start(
                    ktile[:],
                    _r(
                        agout_k[x][base : base + KVF].rearrange(
                            "(dc p n) -> p dc n", p=128, n=NL
                        )
                    ),
                )
                vtile = attnp.tile(
                    [128, NL // 128, D + 2], F32R, tag="vt", bufs=2, name=f"vt{r}"
                )
                nc.sync.dma_start(
                    vtile[:, :, 0:D],
                    _r(
                        agout_v[x][base : base + KVF].rearrange(
                            "(kc p d) -> p kc d", p=128, d=D
                        )
                    ),
                )
                nc.sync.dma_start(
                    vtile[:, :, D : D + 2],
                    _r(vpadd.ap().rearrange("p (kc c) -> p kc c", c=2)),
                )
                for qb in range(NB):
                    qs = slice(qb * 512, (qb + 1) * 512)
                    exps = []
                    for kt_i in range(NL // 128):
                        ps = psum.tile(
                            [128, 512], dt.float32, tag="mm", bufs=4, name="ps_s"
                        )
                        for dc in range(DC):
                            nc.tensor.matmul(
                                ps[:],
                                ktile[:, dc, kt_i * 128 : (kt_i + 1) * 128],
                                qT[x][:, dc, qs],
                                start=(dc == 0),
                                stop=(dc == DC - 1),
                            )
                        ex = attnp.tile(
                            [128, 512], F32R, tag="exp", bufs=12, name=f"ex{kt_i}"
                        )
                        nc.scalar.activation(ex[:], ps[:], AF.Exp)
                        exps.append(ex)
                    for qt_i in range(4):
                        qsl = slice(qt_i * 128, (qt_i + 1) * 128)
                        p1 = psum.tile(
                            [128, 256], dt.float32, tag="po1", bufs=2, name="po1"
                        )
                        p2 = psum.tile(
                            [128, 258], dt.float32, tag="po2", bufs=2, name="po2"
                        )
                        for kc in range(NL // 128):
                            nc.tensor.matmul(
                                p1[:],
                                exps[kc][:, qsl],
                                vtile[:, kc, 0:256],
                                start=(kc == 0),
                                stop=(kc == NL // 128 - 1),
                            )
                            nc.tensor.matmul(
                                p2[:],
                                exps[kc][:, qsl],
                                vtile[:, kc, 256 : D + 2],
                                start=(kc == 0),
                                stop=(kc == NL // 128 - 1),
                            )
                        if r == 0:
                            acc = attnp.tile(
                                [128, D + 2], dt.float32, tag="acc", bufs=8,
                                name=f"acc{qb}{qt_i}",
                            )
                            accs[(qb, qt_i)] = acc
                            nc.vector.tensor_copy(acc[:, 0:256], p1[:])
                            nc.vector.tensor_copy(acc[:, 256 : D + 2], p2[:])
                        else:
                            acc = accs[(qb, qt_i)]
                            nc.vector.tensor_tensor(
                                acc[:, 0:256], acc[:, 0:256], p1[:], ALU.add
                            )
                            nc.vector.tensor_tensor(
                                acc[:, 256 : D + 2], acc[:, 256 : D + 2], p2[:], ALU.add
                            )
            # finalize: out = acc[:, :512] / acc[:, 512] + bv
            for qb in range(NB):
                for qt_i in range(4):
                    acc = accs[(qb, qt_i)]
                    rr = attnp.tile([128, 1], dt.float32, tag="rr", bufs=4, name="rr")
                    nc.vector.reciprocal(rr[:], acc[:, D : D + 1])
                    ot = attnp.tile([128, D], dt.float32, tag="ot", bufs=3, name="ot")
                    nc.vector.tensor_scalar(ot[:], acc[:, 0:D], rr[:], None, ALU.mult)
                    nc.vector.tensor_tensor(ot[:], ot[:], bv[:], ALU.add)
                    r0 = qb * 512 + qt_i * 128
                    nc.sync.dma_start(
                        out_d.ap()[r0 : r0 + 128, col * D : (col + 1) * D], ot[:]
                    )
        attnp.release()
        qtp.release()
        const.release()
        dram.release()
        psum.release()

    nc.compile()
    return nc


_NC = None


def _get_nc():
    global _NC
    if _NC is None:
        _NC = build()
    return _NC


def _chunk_w(w):
    """[X, Y] -> [128, X//128, Y] partition-chunked, contiguous."""
    x, y = w.shape
    return np.ascontiguousarray(w.reshape(x // 128, 128, y).transpose(1, 0, 2))


def _chunk_b(b):
    return np.ascontiguousarray(np.asarray(b, np.float32).reshape(-1, 128).T)


def prep_in_maps(za, zb, W1, b1, W2, b2, Wq, bq, Wk, bk, Wv, bv):
    za = np.asarray(za, np.float32)
    zb = np.asarray(zb, np.float32)
    W1 = np.asarray(W1, np.float32)
    W2 = np.asarray(W2, np.float32)
    Wq = np.asarray(Wq, np.float32)
    Wk = np.asarray(Wk, np.float32)
    Wv = np.asarray(Wv, np.float32)
    b1 = np.asarray(b1, np.float32)
    b2 = np.asarray(b2, np.float32)
    bq = np.asarray(bq, np.float32)
    bk = np.asarray(bk, np.float32)
    bv = np.asarray(bv, np.float32)

    shared = {
        "W1t": _chunk_w(W1),
        "W2t": _chunk_w(W2),
        "Wqt": _chunk_w(Wq / SCALE),
        "Wkt": _chunk_w(Wk),
        "Wvt": _chunk_w(Wv),
        "b1t": _chunk_b(b1),
        "b1p1t": _chunk_b(b1 + 1.0),
        "b2t": _chunk_b(b2 - W2.sum(axis=0)),
        "bqt": _chunk_b(bq / SCALE),
        "bkt": _chunk_b(bk),
        "bvt": np.ascontiguousarray(np.broadcast_to(bv, (128, D)).astype(np.float32)),
        "vpad": np.ascontiguousarray(
            np.broadcast_to(np.tile(np.array([1.0, 0.0], np.float32), HC), (128, 2 * HC))
        ),
    }
    zaT = np.ascontiguousarray(za.T)  # [H, N]
    zbT = np.ascontiguousarray(zb.T)
    in_maps = []
    for c in range(R):
        cs = slice(c * NL, (c + 1) * NL)
        in_maps.append(
            {
                "zaT": _chunk_w(zaT[:, cs]),
                "zbT": _chunk_w(zbT[:, cs]),
                **shared,
            }
        )
    return in_maps


def kernel(**inputs) -> np.ndarray:
    nc = _get_nc()
    in_maps = prep_in_maps(**inputs)
    res = run_bass_kernel_spmd(nc, in_maps, core_ids=list(range(R)))
    return np.concatenate([res.results[c]["out"] for c in range(R)], axis=0)


# revision 4
# speedup vs baseline: 1.2612x; 1.0044x over previous
"""Trainium2 8-core kernel for the Contrast module:

    za_p = ELU(za @ W1 + b1) @ W2 + b2          (same for zb)
    za_ca = softmax((za_p Wq + bq)(zb_p Wk + bk)^T / sqrt(256)) @ (zb_p Wv + bv)
    zb_ca = softmax((zb_p Wq + bq)(za_p Wk + bk)^T / sqrt(256)) @ (za_p Wv + bv)
    out = concat(za_ca, zb_ca, axis=1)

Sharding: rows (N axis) data-parallel across 8 cores; weights replicated.
Each core projects its 1024-row slice of za and zb, computes its K/V
shards, AllGathers K/V (2 collectives, overlapped with compute), and runs
its [1024, 8192] attention block for both directions.

Layout notes:
 - All activations flow feature-major ("transposed"): inputs arrive as
   zaT [h, n] so every matmul contracts over the partition axis without
   any on-chip transposes.  out = lhsT.T @ rhs with
     hT = W1^T zaT, pT = W2^T hT, QT/KT = W^T pT (feature-major)
     V  = pT as lhsT with Wv as rhs (token-major)
     scoresT[k, q] = (KT slice)^T @ QT, attn@V = expT slice as lhsT, V as rhs.
 - f32r (FP32 data, FP22 multiply) matmuls: full PE rate at moving dim>=256.
 - softmax denominator: V panels carry a ones column; attn@V is split into
   N=256 and N=258 matmuls so the rowsum accumulates in PSUM col 512.
 - No max-subtraction in softmax: scores are ~N(0, 0.85^2), exp is safe.
 - ELU+1 = max(x+1, min(exp(x), 1)); the -1 is folded into b2 on the host.
 - 1/16 score scale folded into Wq/bq on the host.
"""

import numpy as np

import concourse.mybir as mybir
import concourse.tile as tile
from concourse import bacc
from concourse.bass_utils import run_bass_kernel_spmd

dt = mybir.dt
AF = mybir.ActivationFunctionType
ALU = mybir.AluOpType

R = 8            # cores
N = 8192         # total rows
H = 1024         # hidden
D = 512          # attention dim
NL = N // R      # rows per core
HC = H // 128    # 8 h-chunks
DC = D // 128    # 4 d-chunks
NB = NL // 512   # 2 n-blocks per core slice
SCALE = 16.0     # sqrt(512/2)
KVF = D * NL     # floats per K (or V) shard

F32R = dt.float32r


def _r(ap):
    return ap.bitcast(F32R)


def build():
    nc = bacc.Bacc("TRN2", target_bir_lowering=False, debug=False, num_devices=R)

    def inp(name, shape):
        return nc.dram_tensor(name, shape, dt.float32, kind="ExternalInput")

    zT = {"a": inp("zaT", [128, HC, NL]), "b": inp("zbT", [128, HC, NL])}
    w1 = inp("W1t", [128, HC, H])
    w2 = inp("W2t", [128, HC, D])
    wq = inp("Wqt", [128, DC, D])
    wk = inp("Wkt", [128, DC, D])
    wv = inp("Wvt", [128, DC, D])
    b1d = inp("b1t", [128, HC])
    b1p1d = inp("b1p1t", [128, HC])
    b2d = inp("b2t", [128, DC])
    bqd = inp("bqt", [128, DC])
    bkd = inp("bkt", [128, DC])
    bvd = inp("bvt", [128, D])
    vpadd = inp("vpad", [128, 2 * HC])
    out_d = nc.dram_tensor("out", [NL, 2 * D], dt.float32, kind="ExternalOutput")

    with tile.TileContext(nc) as tc:
        psum = tc.alloc_tile_pool(name="psum", bufs=1, space="PSUM")
        dram = tc.alloc_tile_pool(name="dram", bufs=1, space="DRAM")
        const = tc.alloc_tile_pool(name="const", bufs=1)
        qtp = tc.alloc_tile_pool(name="qtp", bufs=1)
        wkvp = tc.alloc_tile_pool(name="wkvp", bufs=1)
        projp = tc.alloc_tile_pool(name="projp", bufs=1)

        # ---- constants ----
        b1 = const.tile([128, HC], dt.float32, name="b1")
        b1p1 = const.tile([128, HC], dt.float32, name="b1p1")
        b2 = const.tile([128, DC], dt.float32, name="b2")
        bq = const.tile([128, DC], dt.float32, name="bq")
        bk = const.tile([128, DC], dt.float32, name="bk")
        bv = const.tile([128, D], dt.float32, name="bv")
        for t, d_ in ((b1, b1d), (b1p1, b1p1d), (b2, b2d), (bq, bqd), (bk, bkd), (bv, bvd)):
            nc.sync.dma_start(t[:], d_.ap())
        wqt = wkvp.tile([128, DC, D], F32R, name="wqt")
        wkt = wkvp.tile([128, DC, D], F32R, name="wkt")
        wvt = wkvp.tile([128, DC, D], F32R, name="wvt")
        for t, d_ in ((wqt, wq), (wkt, wk), (wvt, wv)):
            nc.sync.dma_start(t[:], _r(d_.ap()))

        # ---- projection weights (chunked DMA so PE can start early) ----
        w1t = projp.tile([128, HC, H], F32R, name="w1t")
        for hc in range(HC):
            nc.sync.dma_start(w1t[:, hc, :], _r(w1.ap()[:, hc, :]))
        w2t = projp.tile([128, HC, D], F32R, name="w2t")
        nc.sync.dma_start(w2t[:], _r(w2.ap()))

        pT = {
            "a": wkvp.tile([128, DC, NL], F32R, name="pta"),
            "b": wkvp.tile([128, DC, NL], F32R, name="ptb"),
        }

        # AG buffers: direction X's attention consumes K/V derived from the
        # OTHER projection; ag_for[X] is filled from pT[other(X)].
        agin_k = {}
        agin_v = {}
        agout_k = {}
        agout_v = {}
        for x in ("b", "a"):
            agin_k[x] = dram.tile([KVF], dt.float32, name=f"agink_{x}")
            agin_v[x] = dram.tile([KVF], dt.float32, name=f"aginv_{x}")
            agout_k[x] = dram.tile(
                [R * KVF], dt.float32, name=f"agoutk_{x}", addr_space="Shared"
            )
            agout_v[x] = dram.tile(
                [R * KVF], dt.float32, name=f"agoutv_{x}", addr_space="Shared"
            )

        # ================= projection + K/V shards =================
        for src, other in (("a", "b"), ("b", "a")):
            for nb in range(NB):
                ns = slice(nb * 512, (nb + 1) * 512)
                z = projp.tile([128, HC, 512], F32R, tag="z", bufs=2, name=f"z_{src}{nb}")
                nc.sync.dma_start(z[:], _r(zT[src].ap()[:, :, ns]))
                hT = projp.tile([128, HC, 512], F32R, tag="h", bufs=1, name=f"h_{src}{nb}")
                for d1c in range(HC):
                    ps = psum.tile([128, 512], dt.float32, tag="mm", bufs=4, name="ps_h")
                    for hc in range(HC):
                        nc.tensor.matmul(
                            ps[:],
                            w1t[:, hc, d1c * 128 : (d1c + 1) * 128],
                            z[:, hc, :],
                            start=(hc == 0),
                            stop=(hc == HC - 1),
                        )
                    # ELU(x)+1 = max(x+1, min(exp(x), 1)), x = ps + b1
                    e = projp.tile([128, 512], dt.float32, tag="e", bufs=2, name="e")
                    nc.scalar.activation(e[:], ps[:], AF.Exp, bias=b1[:, d1c : d1c + 1])
                    xp1 = projp.tile([128, 512], dt.float32, tag="xp1", bufs=2, name="xp1")
                    nc.vector.tensor_scalar(
                        xp1[:], ps[:], b1p1[:, d1c : d1c + 1], None, ALU.add
                    )
                    nc.vector.tensor_scalar(e[:], e[:], 1.0, None, ALU.min)
                    nc.vector.tensor_tensor(hT[:, d1c, :], xp1[:], e[:], ALU.max)
                for d2c in range(DC):
                    ps = psum.tile([128, 512], dt.float32, tag="mm", bufs=4, name="ps_p")
                    for d1c in range(HC):
                        nc.tensor.matmul(
                            ps[:],
                            w2t[:, d1c, d2c * 128 : (d2c + 1) * 128],
                            hT[:, d1c, :],
                            start=(d1c == 0),
                            stop=(d1c == HC - 1),
                        )
                    nc.scalar.activation(
                        pT[src][:, d2c, ns], ps[:], AF.Identity, bias=b2[:, d2c : d2c + 1]
                    )

            # K/V shards for the *other* direction, written to AG input
            ktv = agin_k[other][:].rearrange("(d n) -> d n", n=NL)
            vv = agin_v[other][:].rearrange("(n d) -> n d", d=D)
            for dc in range(DC):
                for nb in range(NB):
                    ps = psum.tile([128, 512], dt.float32, tag="mm", bufs=4, name="ps_k")
                    for d2c in range(DC):
                        nc.tensor.matmul(
                            ps[:],
                            wkt[:, d2c, dc * 128 : (dc + 1) * 128],
                            pT[src][:, d2c, nb * 512 : (nb + 1) * 512],
                            start=(d2c == 0),
                            stop=(d2c == DC - 1),
                        )
                    s = projp.tile([128, 512], dt.float32, tag="stg", bufs=3, name="stg_k")
                    nc.scalar.activation(s[:], ps[:], AF.Identity, bias=bk[:, dc : dc + 1])
                    nc.sync.dma_start(
                        ktv[dc * 128 : (dc + 1) * 128, nb * 512 : (nb + 1) * 512], s[:]
                    )
            nc.gpsimd.collective_compute(
                "AllGather",
                ALU.bypass,
                ins=[agin_k[other].opt()],
                outs=[agout_k[other].opt()],
                replica_groups=[list(range(R))],
            )
            for nt in range(NL // 128):
                ps = psum.tile([128, 512], dt.float32, tag="mm", bufs=4, name="ps_v")
                for d2c in range(DC):
                    nc.tensor.matmul(
                        ps[:],
                        pT[src][:, d2c, nt * 128 : (nt + 1) * 128],
                        wvt[:, d2c, :],
                        start=(d2c == 0),
                        stop=(d2c == DC - 1),
                    )
                s = projp.tile([128, 512], dt.float32, tag="stg", bufs=3, name="stg_v")
                nc.scalar.activation(s[:], ps[:], AF.Copy)
                nc.sync.dma_start(vv[nt * 128 : (nt + 1) * 128, :], s[:])
            nc.gpsimd.collective_compute(
                "AllGather",
                ALU.bypass,
                ins=[agin_v[other].opt()],
                outs=[agout_v[other].opt()],
                replica_groups=[list(range(R))],
            )

        projp.release()

        # ================= queries =================
        qT = {}
        for x in ("b", "a"):
            qT[x] = qtp.tile([128, DC, NL], F32R, name=f"qt_{x}")
            for dc in range(DC):
                for nb in range(NB):
                    ps = psum.tile([128, 512], dt.float32, tag="mm", bufs=4, name="ps_q")
                    for d2c in range(DC):
                        nc.tensor.matmul(
                            ps[:],
                            wqt[:, d2c, dc * 128 : (dc + 1) * 128],
                            pT[x][:, d2c, nb * 512 : (nb + 1) * 512],
                            start=(d2c == 0),
                            stop=(d2c == DC - 1),
                        )
                    nc.scalar.activation(
                        qT[x][:, dc, nb * 512 : (nb + 1) * 512],
                        ps[:],
                        AF.Identity,
                        bias=bq[:, dc : dc + 1],
                    )
        wkvp.release()

        # ================= attention =================
        attnp = tc.alloc_tile_pool(name="attnp", bufs=1)
        for x, col in (("b", 1), ("a", 0)):
            accs = {}
            for r in range(R):
                base = r * KVF
                ktile = attnp.tile([128, DC, NL], F32R, tag="kt", bufs=2, name=f"kt{r}")
                nc.sync.dma_start(
                    ktile[:],
                    _r(
                        agout_k[x][base : base + KVF].rearrange(
                            "(dc p n) -> p dc n", p=128, n=NL
                        )
                    ),
                )
                vtile = attnp.tile(
                    [128, NL // 128, D + 2], F32R, tag="vt", bufs=2, name=f"vt{r}"
                )
                nc.sync.dma_start(
                    vtile[:, :, 0:D],
                    _r(
                        agout_v[x][base : base + KVF].rearrange(
                            "(kc p d) -> p kc d", p=128, d=D
                        )
                    ),
                )
                nc.sync.dma_start(
                    vtile[:, :, D : D + 2],
                    _r(vpadd.ap().rearrange("p (kc c) -> p kc c", c=2)),
                )
                for qb in range(NB):
                    qs = slice(qb * 512, (qb + 1) * 512)
                    exps = []
                    for kt_i in range(NL // 128):
                        ps = psum.tile(
                            [128, 512], dt.float32, tag="mm", bufs=4, name="ps_s"
                        )
                        for dc in range(DC):
                            nc.tensor.matmul(
                                ps[:],
                                ktile[:, dc, kt_i * 128 : (kt_i + 1) * 128],
                                qT[x][:, dc, qs],
                                start=(dc == 0),
                                stop=(dc == DC - 1),
                            )
                        ex = attnp.tile(
                            [128, 512], F32R, tag="exp", bufs=32, name=f"ex{kt_i}"
                        )
                        nc.scalar.activation(ex[:], ps[:], AF.Exp)
                        exps.append(ex)
                    for qt_i in range(4):
                        qsl = slice(qt_i * 128, (qt_i + 1) * 128)
                        p1 = psum.tile(
                            [128, 256], dt.float32, tag="po1", bufs=2, name="po1"
                        )
                        p2 = psum.tile(
                            [128, 258], dt.float32, tag="po2", bufs=2, name="po2"
                        )
                        for kc in range(NL // 128):
                            nc.tensor.matmul(
                                p1[:],
                                exps[kc][:, qsl],
                                vtile[:, kc, 0:256],
                                start=(kc == 0),
                                stop=(kc == NL // 128 - 1),
                            )
                            nc.tensor.matmul(
                                p2[:],
                                exps[kc][:, qsl],
                                vtile[:, kc, 256 : D + 2],
                                start=(kc == 0),
                                stop=(kc == NL // 128 - 1),
                            )
                        if r == 0:
                            acc = attnp.tile(
                                [128, D + 2], dt.float32, tag="acc", bufs=8,
                                name=f"acc{qb}{qt_i}",
                            )
                            accs[(qb, qt_i)] = acc
                            nc.vector.tensor_copy(acc[:, 0:256], p1[:])
                            nc.vector.tensor_copy(acc[:, 256 : D + 2], p2[:])
                        else:
                            acc = accs[(qb, qt_i)]
                            nc.vector.tensor_tensor(
                                acc[:, 0:256], acc[:, 0:256], p1[:], ALU.add
                            )
                            nc.vector.tensor_tensor(
                                acc[:, 256 : D + 2], acc[:, 256 : D + 2], p2[:], ALU.add
                            )
            # finalize: out = acc[:, :512] / acc[:, 512] + bv
            for qb in range(NB):
                for qt_i in range(4):
                    acc = accs[(qb, qt_i)]
                    rr = attnp.tile([128, 1], dt.float32, tag="rr", bufs=4, name="rr")
                    nc.vector.reciprocal(rr[:], acc[:, D : D + 1])
                    ot = attnp.tile([128, D], dt.float32, tag="ot", bufs=3, name="ot")
                    nc.vector.tensor_scalar(ot[:], acc[:, 0:D], rr[:], None, ALU.mult)
                    nc.vector.tensor_tensor(ot[:], ot[:], bv[:], ALU.add)
                    r0 = qb * 512 + qt_i * 128
                    nc.sync.dma_start(
                        out_d.ap()[r0 : r0 + 128, col * D : (col + 1) * D], ot[:]
                    )
        attnp.release()
        qtp.release()
        const.release()
        dram.release()
        psum.release()

    nc.compile()
    return nc


_NC = None


def _get_nc():
    global _NC
    if _NC is None:
        _NC = build()
    return _NC


def _chunk_w(w):
    """[X, Y] -> [128, X//128, Y] partition-chunked, contiguous."""
    x, y = w.shape
    return np.ascontiguousarray(w.reshape(x // 128, 128, y).transpose(1, 0, 2))


def _chunk_b(b):
    return np.ascontiguousarray(np.asarray(b, np.float32).reshape(-1, 128).T)


def prep_in_maps(za, zb, W1, b1, W2, b2, Wq, bq, Wk, bk, Wv, bv):
    za = np.asarray(za, np.float32)
    zb = np.asarray(zb, np.float32)
    W1 = np.asarray(W1, np.float32)
    W2 = np.asarray(W2, np.float32)
    Wq = np.asarray(Wq, np.float32)
    Wk = np.asarray(Wk, np.float32)
    Wv = np.asarray(Wv, np.float32)
    b1 = np.asarray(b1, np.float32)
    b2 = np.asarray(b2, np.float32)
    bq = np.asarray(bq, np.float32)
    bk = np.asarray(bk, np.float32)
    bv = np.asarray(bv, np.float32)

    shared = {
        "W1t": _chunk_w(W1),
        "W2t": _chunk_w(W2),
        "Wqt": _chunk_w(Wq / SCALE),
        "Wkt": _chunk_w(Wk),
        "Wvt": _chunk_w(Wv),
        "b1t": _chunk_b(b1),
        "b1p1t": _chunk_b(b1 + 1.0),
        "b2t": _chunk_b(b2 - W2.sum(axis=0)),
        "bqt": _chunk_b(bq / SCALE),
        "bkt": _chunk_b(bk),
        "bvt": np.ascontiguousarray(np.broadcast_to(bv, (128, D)).astype(np.float32)),
        "vpad": np.ascontiguousarray(
            np.broadcast_to(np.tile(np.array([1.0, 0.0], np.float32), HC), (128, 2 * HC))
        ),
    }
    zaT = np.ascontiguousarray(za.T)  # [H, N]
    zbT = np.ascontiguousarray(zb.T)
    in_maps = []
    for c in range(R):
        cs = slice(c * NL, (c + 1) * NL)
        in_maps.append(
            {
                "zaT": _chunk_w(zaT[:, cs]),
                "zbT": _chunk_w(zbT[:, cs]),
                **shared,
            }
        )
    return in_maps


def kernel(**inputs) -> np.ndarray:
    nc = _get_nc()
    in_maps = prep_in_maps(**inputs)
    res = run_bass_kernel_spmd(nc, in_maps, core_ids=list(range(R)))
    return np.concatenate([res.results[c]["out"] for c in range(R)], axis=0)
